# revision 21
# baseline (speedup 1.0000x reference)
"""Trainium2 Bass kernel for nn_CannyFilter_BPDA (batch-parallel over 8 cores).

Self-contained: builds a Bass/Tile program per core processing 4 images
(B=32 total / 8 cores), runs via a cached jit(shard_map(bass_exec)) on the
8 axon devices, gathers output.

Host<->device wire is the bottleneck (~45 MB/s through the axon relay), so
the IO contract is minimized:
  - input: channels are summed on the host (the kernel only ever uses
    I = sum_c img_c), so 33.5 MB crosses the wire instead of 100 MB.
  - constants + the dummy output-donation buffer live on device, put once.
  - output: {0,1} edge map is bit-packed on device (8 pixels/byte) and
    expanded with np.unpackbits on the host -> 1 MB instead of 33.5 MB.
  - the jit executable is built once per process and cached.

Algorithm (scale-invariant reformulation of the reference; outputs depend
only on comparisons, so a positive global scale lambda on gx/gy is folded
into thresholds):
  I   = sum_c img_c                    (host)
  Gv  = rho*(I_up + I_dn) + I          (vertical gaussian tap, edge pad)
  Bl  = rho*(Gv_l + Gv_r) + Gv         (horizontal gaussian tap)
  vx  = 0.5*Bl_up + Bl + 0.5*Bl_dn     (PE band matmul, exact 0.5/1 weights)
  wy  = Bl_dn - Bl_up                  (PE band matmul)
  gx  = vx_r - vx_l ; gy = 0.5*(wy_l + wy_r) + wy
  m2  = gx^2 + gy^2 ; mag = sqrt(m2)
  orientation class via |gy| vs tan(k*22.5 deg)*|gx| comparisons
  NMS: keep iff mag > max(neighbor pair along class), neighbors via PE
       shift matmuls (exact fp32r permutation rows)
  z   = keep * ((mag > 0.1*lam) + (mag > 0.3*lam))   in {0,1,2} (= 2*thin)
  hv  = 3x3 box sum of z (exact small ints, bf16/PE)
  out = (z == 2) + (hv >= 2) * (z == 1)   in {0,1}, bit-packed 8px/byte
"""

import math
import os
import sys
from concurrent.futures import ThreadPoolExecutor

for _p in ("/opt/trn_rl_repo", "/opt/pypackages"):
    if os.path.isdir(_p) and _p not in sys.path:
        sys.path.append(_p)

import numpy as np

import concourse.bass as bass
import concourse.bacc as bacc
import concourse.tile as tile
from concourse import mybir
from concourse._compat import with_exitstack
from concourse.mybir import AluOpType as alu

F32 = mybir.dt.float32
U8 = mybir.dt.uint8
F32R = mybir.dt.float32r
BF16 = mybir.dt.bfloat16
AF = mybir.ActivationFunctionType

N_CORES = 8
BPC = 4          # images per core (total)
CPB = 4          # images per core per NEFF call; BPC//CPB pipelined calls
                 # (CPB=2 pipelining tested: the relay serializes transfers
                 # and exec dispatches, so chunking only added latency)
NCHUNK = BPC // CPB
C, H, W = 3, 512, 512
P, NT = 128, 4   # partitions, row-subtiles (H = NT*P)
WB = W // 8      # bit-packed output bytes per row

# ---------------------------------------------------------------- constants
RHO = float(np.float32(math.exp(-0.5)))
_B1D = 1.0 / (1.0 + 2.0 * math.exp(-0.5))       # gaussian 1d center weight
LAM = 3.0 / (_B1D * _B1D)                        # my gx = LAM * ref gx
QF = 65535.0 / 3.0   # host ships I as u16 = round(I*QF); pipeline is
                     # scale-invariant so only the thresholds pick up QF
TLV = float(np.float32(0.1 * LAM * QF))
THV = float(np.float32(0.3 * LAM * QF))
T1V = float(np.float32(math.tan(math.radians(11.25))))
T2V = float(np.float32(math.tan(math.radians(33.75))))

# fp32 band-matrix indices
BI_101_TOP, BI_101_MID, BI_101_BOT = 0, 1, 2
BI_VX_TOP, BI_VX_MID, BI_VX_BOT = 3, 4, 5
BI_WY_TOP, BI_WY_MID, BI_WY_BOT = 6, 7, 8
BI_SSU, BI_SSD = 9, 10
# single-entry cross-tile edge matrices (used as K=64 partition slices):
#   TOPFIX_w: M[127, 0] = w  -> out row 0   += w * src[row127 of t-1]
#   BOTFIX_w: M[0, 127] = w  -> out row 127 += w * src[row0   of t+1]
BI_TOPFIX_1, BI_TOPFIX_05, BI_TOPFIX_N1, BI_BOTFIX_1, BI_BOTFIX_05 = 11, 12, 13, 14, 15
N_BANDS = 16
# bf16 band indices
BI16_111, BI16_TOPFIX_1, BI16_BOTFIX_1 = 0, 1, 2
N_BANDS16 = 3


def make_const_arrays():
    """Band matrices M[k, m]: out[m] = sum_k M[k,m] * in[k]."""
    b101 = np.zeros((3, 128, 128), np.float32)
    for v in range(3):
        for m in range(128):
            if m - 1 >= 0:
                b101[v, m - 1, m] += 1.0
            if m + 1 <= 127:
                b101[v, m + 1, m] += 1.0
    b101[0, 0, 0] += 1.0      # top: replicate row -1 -> row 0
    b101[2, 127, 127] += 1.0  # bottom: replicate

    bvx = np.zeros((3, 128, 128), np.float32)
    for v in range(3):
        for m in range(128):
            bvx[v, m, m] += 1.0
            if m - 1 >= 0:
                bvx[v, m - 1, m] += 0.5
            if m + 1 <= 127:
                bvx[v, m + 1, m] += 0.5
    bvx[0, 0, 0] += 0.5
    bvx[2, 127, 127] += 0.5

    bwy = np.zeros((3, 128, 128), np.float32)
    for v in range(3):
        for m in range(128):
            if m - 1 >= 0:
                bwy[v, m - 1, m] += -1.0
            if m + 1 <= 127:
                bwy[v, m + 1, m] += 1.0
    bwy[0, 0, 0] += -1.0      # top: wy[0] = B[1] - B[0]
    bwy[2, 127, 127] += 1.0   # bottom: wy[127] = B[127] - B[126]

    ssu = np.zeros((128, 128), np.float32)  # out[m] = in[m-1], row0 -> 0
    ssd = np.zeros((128, 128), np.float32)  # out[m] = in[m+1], row127 -> 0
    for m in range(1, 128):
        ssu[m - 1, m] = 1.0
    for m in range(0, 127):
        ssd[m + 1, m] = 1.0

    ee = np.zeros((5, 128, 128), np.float32)
    ee[0, 127, 0] = 1.0    # TOPFIX_1
    ee[1, 127, 0] = 0.5    # TOPFIX_05
    ee[2, 127, 0] = -1.0   # TOPFIX_N1
    ee[3, 0, 127] = 1.0    # BOTFIX_1
    ee[4, 0, 127] = 0.5    # BOTFIX_05

    bands = np.concatenate([b101, bvx, bwy, ssu[None], ssd[None], ee], 0)
    assert bands.shape[0] == N_BANDS

    b16 = np.zeros((N_BANDS16, 128, 128), np.float32)
    for m in range(128):
        b16[BI16_111, m, m] = 1.0
        if m - 1 >= 0:
            b16[BI16_111, m - 1, m] = 1.0
        if m + 1 <= 127:
            b16[BI16_111, m + 1, m] = 1.0
    b16[BI16_TOPFIX_1, 127, 0] = 1.0
    b16[BI16_BOTFIX_1, 0, 127] = 1.0
    import ml_dtypes

    return bands, b16.astype(ml_dtypes.bfloat16)


# ------------------------------------------------------- custom DVE ops
_OPS = {}


def register_custom_ops():
    if _OPS:
        return _OPS
    from concourse import dve_ops as dops
    from concourse.dve_spec import (
        C0,
        C1,
        One,
        Spec,
        Src0,
        Src1,
        Zero,
        _has_src1,
        eq,
        lower,
        maxx,
        sq,
    )
    from concourse.dve_uop import DveOpSpec

    def reg(name, body, reference):
        if name in dops._SUB_OPCODE_FOR_NAME:
            return {o.name: o for o in dops.OPS}[name]
        spec = Spec(body=body, reference=reference)
        row = max(dops._SUB_OPCODE_FOR_NAME.values()) + 1
        assert row < 0x20, "out of custom-DVE opcode rows"
        dops._SUB_OPCODE_FOR_NAME[name] = row
        shas = {}
        for ver in ("v3", "v4"):
            try:
                s = DveOpSpec(
                    name=name,
                    opcode=row,
                    uops=lower(spec, ver=ver),
                    rd1_en=_has_src1(spec),
                )
                shas[ver] = s.sha(ver)
            except Exception:
                pass
        op = dops.DveOp(name, spec, subdim=False, uops_sha=shas)
        dops.OPS.append(op)
        dops.CUSTOM_DVE_SPECS[name] = spec
        return op

    f32 = np.float32

    def _f(x):
        return np.asarray(x, np.float32)

    _OPS["MAG2"] = reg(
        "ANT_CANNY_MAG2",
        sq(Src0) + sq(Src1),
        lambda i0, i1, c0, c1, c2: (_f(i0) * _f(i0)) + (_f(i1) * _f(i1)),
    )
    _OPS["U4A"] = reg(
        "ANT_CANNY_U4A",
        ((Src0 * C0) < Src1) + ((Src0 * C1) < Src1),
        lambda i0, i1, c0, c1, c2: (
            (_f(i0) * f32(c0) < _f(i1)).astype(np.float32)
            + (_f(i0) * f32(c1) < _f(i1)).astype(np.float32)
        ),
    )
    _OPS["U4B"] = reg(
        "ANT_CANNY_U4B",
        ((Src0 * C0) > Src1) + ((Src0 * C1) > Src1),
        lambda i0, i1, c0, c1, c2: (
            (_f(i0) * f32(c0) > _f(i1)).astype(np.float32)
            + (_f(i0) * f32(c1) > _f(i1)).astype(np.float32)
        ),
    )
    _OPS["ZC"] = reg(
        "ANT_CANNY_ZC",
        (Src0 < Src1) * ((Src1 > C0) + (Src1 > C1)),
        lambda i0, i1, c0, c1, c2: (_f(i0) < _f(i1)).astype(np.float32)
        * (
            (_f(i1) > f32(c0)).astype(np.float32)
            + (_f(i1) > f32(c1)).astype(np.float32)
        ),
    )
    _OPS["OUT"] = reg(
        "ANT_CANNY_OUT",
        (Src0 > C0) + ((Src1 > C1) * eq(Src0, One)),
        lambda i0, i1, c0, c1, c2: (_f(i0) > f32(c0)).astype(np.float32)
        + (_f(i1) > f32(c1)).astype(np.float32)
        * (_f(i0) == f32(1.0)).astype(np.float32),
    )
    return _OPS


# ------------------------------------------------------------ kernel body
@with_exitstack
def canny_kernel(ctx, tc, img, bands, b16, outp):
    nc = tc.nc
    V, A, G, T = nc.vector, nc.scalar, nc.gpsimd, nc.tensor
    ops = register_custom_ops()

    sp = ctx.enter_context(tc.tile_pool(name="planes", bufs=1))
    cpool = ctx.enter_context(tc.tile_pool(name="consts", bufs=1))
    pp = ctx.enter_context(tc.tile_pool(name="psum", bufs=1, space="PSUM"))

    # ---- load constants into SBUF
    bt = cpool.tile([128, N_BANDS, 128], F32, tag="bands")
    nc.sync.dma_start(bt[:], bands.rearrange("b k m -> k b m"))
    bt16 = cpool.tile([128, N_BANDS16, 128], BF16, tag="bands16")
    nc.sync.dma_start(bt16[:], b16.rearrange("b k m -> k b m"))

    use_f32r = os.environ.get("CANNY_F32R", "0") == "1"

    def band(i):
        a = bt[:, i, :]
        return a.bitcast(F32R) if use_f32r else a

    def r(ap):
        return ap.bitcast(F32R) if use_f32r else ap

    TRI_SEL = {
        "101": (BI_101_TOP, BI_101_MID, BI_101_MID, BI_101_BOT),
        "vx": (BI_VX_TOP, BI_VX_MID, BI_VX_MID, BI_VX_BOT),
        "wy": (BI_WY_TOP, BI_WY_MID, BI_WY_MID, BI_WY_BOT),
    }
    EDGE_W = {  # (topfix matrix, botfix matrix)
        "101": (BI_TOPFIX_1, BI_BOTFIX_1),
        "vx": (BI_TOPFIX_05, BI_BOTFIX_05),
        "wy": (BI_TOPFIX_N1, BI_BOTFIX_1),
    }

    def run_group(psum_tile, mms):
        for i, (lhsT, rhs) in enumerate(mms):
            T.matmul(
                psum_tile[:], lhsT, rhs, start=(i == 0), stop=(i == len(mms) - 1)
            )

    def tri_matmul(psum_tile, kind, src_plane, t):
        """psum_tile[m,:] = band conv of src_plane subtile t incl cross-tile."""
        etop, ebot = EDGE_W[kind]
        mms = [(band(TRI_SEL[kind][t]), r(src_plane[:, t, :]))]
        if t > 0:
            # out row 0 += w * src[127, t-1] (full-K single-entry matrix)
            mms.append((band(etop), r(src_plane[:, t - 1, :])))
        if t < NT - 1:
            # out row 127 += w * src[0, t+1]
            mms.append((band(ebot), r(src_plane[:, t + 1, :])))
        run_group(psum_tile, mms)

    PARITY_TAGS = {"s1", "s2", "s4", "s5", "s8"}
    par = [0]

    def plane(tag, dtype=F32, shape=None):
        if tag in PARITY_TAGS:
            tag = f"{tag}_{par[0]}"
        return sp.tile([P, NT, W] if shape is None else shape, dtype, tag=tag, name=tag)

    for b in range(CPB):
        par[0] = b % 2
        # ---- load pre-summed u16-quantized intensity, convert to f32
        raw = sp.tile([P, NT, W], mybir.dt.uint16, tag="raw", name="raw")
        nc.sync.dma_start(raw[:], img[b].rearrange("(t p) w -> p t w", p=P))
        ii = plane("s5")
        V.tensor_copy(ii[:], raw[:])

        # ---- vertical gaussian tap: tvI = I_up + I_dn (PE), Gv = rho*tvI + I
        gv = plane("s1")
        for t in range(NT):
            tv = pp.tile([P, W], F32, tag="tv", name="tv")
            tri_matmul(tv, "101", ii, t)
            V.scalar_tensor_tensor(
                gv[:, t, :], tv[:], RHO, ii[:, t, :], alu.mult, alu.add
            )

        # ---- horizontal gaussian tap
        th = plane("s2")
        V.tensor_add(th[:, :, 1:511], gv[:, :, 0:510], gv[:, :, 2:512])
        V.tensor_add(th[:, :, 0:1], gv[:, :, 0:1], gv[:, :, 1:2])
        V.tensor_add(th[:, :, 511:512], gv[:, :, 510:511], gv[:, :, 511:512])
        bl = plane("s3")
        V.scalar_tensor_tensor(bl[:], th[:], RHO, gv[:], alu.mult, alu.add)

        # ---- sobel: vx/wy bands on PE, then horizontal parts
        gx = plane("s1")
        gy = plane("s4")
        th2 = plane("s2")
        vxs = plane("s13")
        wys = plane("s14")
        for t in range(NT):
            vx = pp.tile([P, W], F32, tag="vx", name="vx")
            tri_matmul(vx, "vx", bl, t)
            A.copy(vxs[:, t, :], vx[:])
            V.tensor_sub(gx[:, t, 1:511], vxs[:, t, 2:512], vxs[:, t, 0:510])
            V.tensor_sub(gx[:, t, 0:1], vxs[:, t, 1:2], vxs[:, t, 0:1])
            V.tensor_sub(gx[:, t, 511:512], vxs[:, t, 511:512], vxs[:, t, 510:511])
            wy = pp.tile([P, W], F32, tag="wy", name="wy")
            tri_matmul(wy, "wy", bl, t)
            A.copy(wys[:, t, :], wy[:])
            V.tensor_add(th2[:, t, 1:511], wys[:, t, 0:510], wys[:, t, 2:512])
            V.tensor_add(th2[:, t, 0:1], wys[:, t, 0:1], wys[:, t, 1:2])
            V.tensor_add(th2[:, t, 511:512], wys[:, t, 510:511], wys[:, t, 511:512])
            V.scalar_tensor_tensor(
                gy[:, t, :], th2[:, t, :], 0.5, wys[:, t, :], alu.mult, alu.add
            )

        # ---- magnitude
        m2 = plane("s2")
        V._custom_dve(ops["MAG2"], out=m2[:], in0=gx[:], in1=gy[:])
        mag = plane("s5")
        A.activation(mag[:], m2[:], AF.Sqrt)

        # ---- orientation class count u4 in {0..4}
        ax = plane("s8")
        A.activation(ax[:], gx[:], AF.Abs)
        ay = plane("s9")
        A.activation(ay[:], gy[:], AF.Abs)
        u4a = plane("s2")
        V._custom_dve(ops["U4A"], out=u4a[:], in0=ax[:], in1=ay[:], s0=T1V, s1=T2V)
        u4b = plane("s6")
        V._custom_dve(ops["U4B"], out=u4b[:], in0=ay[:], in1=ax[:], s0=T2V, s1=T1V)
        u4 = plane("s7")
        V.tensor_add(u4[:], u4a[:], u4b[:])
        gp = plane("s2")
        V.tensor_mul(gp[:], gx[:], gy[:])
        spm = plane("s6", U8)
        V.tensor_single_scalar(spm[:], gp[:], 0.0, alu.is_gt)
        m1m = plane("s8", U8)
        V.tensor_single_scalar(m1m[:], u4[:], 1.0, alu.is_equal)
        m2m = plane("s9", U8)
        V.tensor_single_scalar(m2m[:], u4[:], 2.0, alu.is_equal)
        m3m = plane("s10", U8)
        V.tensor_single_scalar(m3m[:], u4[:], 3.0, alu.is_equal)

        # ---- NMS neighbor maxes (P0 doubles as NB selection buffer)
        p0 = plane("s2")
        V.tensor_max(p0[:, :, 1:511], mag[:, :, 0:510], mag[:, :, 2:512])
        A.copy(p0[:, :, 0:1], mag[:, :, 1:2])
        A.copy(p0[:, :, 511:512], mag[:, :, 510:511])
        p1 = plane("s11")
        p2t = plane("s7")
        p3 = plane("s12")
        mus = plane("s15")
        for t in range(NT):
            mu = pp.tile([P, W], F32, tag="shU", name="mu", bufs=2)
            mmu = [(band(BI_SSU), r(mag[:, t, :]))]
            if t > 0:  # row 0 of subtile t is mag row 127 of subtile t-1
                mmu.append((band(BI_TOPFIX_1), r(mag[:, t - 1, :])))
            run_group(mu, mmu)
            A.copy(mus[:, t, :], mu[:])
            md = pp.tile([P, W], F32, tag="shD", name="md", bufs=2)
            mmd = [(band(BI_SSD), r(mag[:, t, :]))]
            if t < NT - 1:
                mmd.append((band(BI_BOTFIX_1), r(mag[:, t + 1, :])))
            run_group(md, mmd)
            V.tensor_max(p2t[:, t, :], mus[:, t, :], md[:])
            V.tensor_max(p1[:, t, 1:511], mus[:, t, 2:512], md[:, 0:510])
            A.copy(p1[:, t, 0:1], mus[:, t, 1:2])
            V.tensor_copy(p1[:, t, 511:512], md[:, 510:511])
            V.tensor_max(p3[:, t, 1:511], mus[:, t, 0:510], md[:, 2:512])
            V.tensor_copy(p3[:, t, 0:1], md[:, 1:2])
            A.copy(p3[:, t, 511:512], mus[:, t, 510:511])

        # ---- diagonal pair selection by gradient sign, then NB by class
        pd1 = plane("s13")
        A.copy(pd1[:], p3[:])
        V.copy_predicated(pd1[:], spm[:], p1[:])
        pd3 = plane("s14")
        A.copy(pd3[:], p1[:])
        V.copy_predicated(pd3[:], spm[:], p3[:])
        V.copy_predicated(p0[:], m1m[:], pd1[:])
        V.copy_predicated(p0[:], m2m[:], p2t[:])
        V.copy_predicated(p0[:], m3m[:], pd3[:])

        # ---- NMS keep + double threshold -> z in {0,1,2} (bf16)
        z = plane("z", BF16)
        V._custom_dve(ops["ZC"], out=z[:], in0=p0[:], in1=mag[:], s0=TLV, s1=THV)

        # ---- hysteresis: 3x3 box sum of z
        hr1 = plane("h1", BF16)
        V.tensor_add(hr1[:, :, 1:511], z[:, :, 0:510], z[:, :, 2:512])
        A.copy(hr1[:, :, 0:1], z[:, :, 1:2])
        A.copy(hr1[:, :, 511:512], z[:, :, 510:511])
        hrow = plane("h2", BF16)
        V.tensor_add(hrow[:], hr1[:], z[:])

        outpl = plane("s1")
        for t in range(NT):
            hv = pp.tile([P, W], F32, tag="hv", name="hv")
            mms = [(bt16[:, BI16_111, :], hrow[:, t, :])]
            if t > 0:
                mms.append((bt16[:, BI16_TOPFIX_1, :], hrow[:, t - 1, :]))
            if t < NT - 1:
                mms.append((bt16[:, BI16_BOTFIX_1, :], hrow[:, t + 1, :]))
            run_group(hv, mms)
            V._custom_dve(
                ops["OUT"], out=outpl[:, t, :], in0=z[:, t, :], in1=hv[:], s0=1.5, s1=1.6
            )

        # ---- bit-pack 8 px/byte (little-endian) and store
        v2 = outpl[:].rearrange("p t (a b) -> p t a b", b=2)
        pk1 = plane("pk1", F32, shape=[P, NT, 256])
        V.scalar_tensor_tensor(
            pk1[:], v2[:, :, :, 1], 2.0, v2[:, :, :, 0], alu.mult, alu.add
        )
        v4 = pk1[:].rearrange("p t (a b) -> p t a b", b=2)
        pk2 = plane("pk2", F32, shape=[P, NT, 128])
        V.scalar_tensor_tensor(
            pk2[:], v4[:, :, :, 1], 4.0, v4[:, :, :, 0], alu.mult, alu.add
        )
        v8 = pk2[:].rearrange("p t (a b) -> p t a b", b=2)
        pk3 = plane("pk3", U8, shape=[P, NT, WB])
        V.scalar_tensor_tensor(
            pk3[:], v8[:, :, :, 1], 16.0, v8[:, :, :, 0], alu.mult, alu.add
        )
        nc.sync.dma_start(outp[b].rearrange("(t p) w -> p t w", p=P), pk3[:])


# --------------------------------------------------------------- build + run
_ENGINE = {}


def build_nc():
    if "nc" in _ENGINE:
        return _ENGINE["nc"]
    register_custom_ops()
    nc = bacc.Bacc("TRN2", target_bir_lowering=False, debug=False)
    img = nc.dram_tensor("img", [CPB, H, W], mybir.dt.uint16, kind="ExternalInput").ap()
    bands = nc.dram_tensor("bands", [N_BANDS, 128, 128], F32, kind="ExternalInput").ap()
    b16 = nc.dram_tensor("b16", [N_BANDS16, 128, 128], BF16, kind="ExternalInput").ap()
    outp = nc.dram_tensor("out", [CPB, H, WB], U8, kind="ExternalOutput").ap()
    with tile.TileContext(nc) as tc:
        canny_kernel(tc, img, bands, b16, outp)
    nc.compile()
    _ENGINE["nc"] = nc
    return nc


def _build_engine():
    """Build (once) the jitted 8-core executable + device-resident operands."""
    if "fn" in _ENGINE:
        return _ENGINE

    import jax
    from jax.sharding import Mesh, NamedSharding, PartitionSpec

    try:
        from jax import shard_map as _shard_map

        def shard_map(f, mesh, in_specs, out_specs):
            return _shard_map(
                f, mesh=mesh, in_specs=in_specs, out_specs=out_specs, check_vma=False
            )
    except ImportError:
        from jax.experimental.shard_map import shard_map as _shard_map_exp

        def shard_map(f, mesh, in_specs, out_specs):
            return _shard_map_exp(
                f, mesh=mesh, in_specs=in_specs, out_specs=out_specs, check_rep=False
            )

    from concourse.bass2jax import (
        _bass_exec_p,
        install_neuronx_cc_hook,
        partition_id_tensor,
    )

    install_neuronx_cc_hook()
    nc = build_nc()

    partition_name = nc.partition_id_tensor.name if nc.partition_id_tensor else None
    in_names, out_names, out_avals = [], [], []
    for alloc in nc.m.functions[0].allocations:
        if not isinstance(alloc, mybir.MemoryLocationSet):
            continue
        name = alloc.memorylocations[0].name
        if alloc.kind == "ExternalInput":
            if name != partition_name:
                in_names.append(name)
        elif alloc.kind == "ExternalOutput":
            out_names.append(name)
            out_avals.append(
                jax.core.ShapedArray(
                    tuple(alloc.tensor_shape), mybir.dt.np(alloc.dtype)
                )
            )
    assert in_names == ["img", "bands", "b16"] and out_names == ["out"], (
        in_names,
        out_names,
    )
    all_in_names = in_names + out_names
    if partition_name is not None:
        all_in_names.append(partition_name)

    def _body(*args):
        operands = list(args)
        if partition_name is not None:
            operands.append(partition_id_tensor())
        outs = _bass_exec_p.bind(
            *operands,
            out_avals=tuple(out_avals),
            in_names=tuple(all_in_names),
            out_names=tuple(out_names),
            lowering_input_output_aliases=(),
            sim_require_finite=True,
            sim_require_nnan=True,
            nc=nc,
        )
        return tuple(outs)

    devices = jax.devices()[:N_CORES]
    assert len(devices) >= N_CORES
    mesh = Mesh(np.asarray(devices), ("core",))
    sh = NamedSharding(mesh, PartitionSpec("core"))
    n_args = len(in_names) + len(out_names)

    def _make_jit():
        return jax.jit(
            shard_map(
                _body,
                mesh=mesh,
                in_specs=(PartitionSpec("core"),) * n_args,
                out_specs=(PartitionSpec("core"),) * len(out_names),
            ),
            keep_unused=True,
        )

    import ml_dtypes

    arg_structs = [
        jax.ShapeDtypeStruct((N_CORES * CPB, H, W), np.uint16, sharding=sh),
        jax.ShapeDtypeStruct((N_CORES * N_BANDS, 128, 128), np.float32, sharding=sh),
        jax.ShapeDtypeStruct(
            (N_CORES * N_BANDS16, 128, 128), ml_dtypes.bfloat16, sharding=sh
        ),
        jax.ShapeDtypeStruct((N_CORES * CPB, H, WB), np.uint8, sharding=sh),
    ]
    try:
        # AOT compile with the bass effect suppressed: C++ fast-path
        # dispatch + no per-call jit cache lookup (real wall time on
        # this 1-vCPU host)
        from concourse.bass2jax import fast_dispatch_compile

        fn = fast_dispatch_compile(lambda: _make_jit().lower(*arg_structs).compile())
    except Exception:
        fn = _make_jit()

    bands, b16 = make_const_arrays()
    bands_d = jax.device_put(np.concatenate([bands] * N_CORES, 0), sh)
    b16_d = jax.device_put(np.concatenate([b16] * N_CORES, 0), sh)
    # dummy operands for the "out" ExternalOutput slot (kernel writes every
    # byte, so contents never matter); one per in-flight chunk, put once
    outdummy_d = [
        jax.device_put(np.zeros((N_CORES * CPB, H, WB), np.uint8), sh)
        for _ in range(NCHUNK)
    ]
    jax.block_until_ready((bands_d, b16_d, outdummy_d))

    _ENGINE.update(
        fn=fn,
        sh=sh,
        devices=devices,
        bands_d=bands_d,
        b16_d=b16_d,
        outdummy_d=outdummy_d,
        jax=jax,
        put_pool=ThreadPoolExecutor(N_CORES),
        fetch_pool=ThreadPoolExecutor(N_CORES * NCHUNK),
    )
    return _ENGINE


_LUT = []


def _bit_lut():
    """[256, 8] f32: byte value -> its 8 bits (little-endian) as floats."""
    if not _LUT:
        bits = np.unpackbits(
            np.arange(256, dtype=np.uint8)[:, None], axis=1, bitorder="little"
        )
        _LUT.append(np.ascontiguousarray(bits.astype(np.float32)))
    return _LUT[0]


def _quantize(img, c, k):
    """u16 quantized I = floor((sum_c img)*QF) for core c, chunk k.

    3 memory passes (add, add, fused mul+unsafe-cast). Floor instead of
    round shifts the quantization noise mean by step/2 — a constant offset
    on I that the gradient operators cancel, so output flips are unchanged
    in magnitude. Minimizing host passes matters: this box has 1 vCPU
    shared with the websocket proxy that carries the device transfers.
    """
    g0 = c * BPC + k * CPB
    sl = img[g0 : g0 + CPB]
    s = sl[:, 0] + sl[:, 1]
    s += sl[:, 2]
    q = np.empty(s.shape, np.uint16)
    np.multiply(s, np.float32(QF), out=q, casting="unsafe")
    return q


def kernel(**inputs):
    img = np.asarray(inputs["img"])
    assert img.shape == (N_CORES * BPC, C, H, W), img.shape
    if img.dtype != np.float32:
        img = img.astype(np.float32)
    eng = _build_engine()
    jax = eng["jax"]
    devices = eng["devices"]
    fn = eng["fn"]

    out = np.empty((N_CORES * BPC, 1, H, W), np.float32)
    fetch_pool = eng["fetch_pool"]
    futs = []
    for k in range(NCHUNK):
        # sequential per-core quantize+put: on this 1-vCPU host, threads
        # parallelize nothing — front-load the numpy work core by core so
        # each put's async streaming proceeds while the next core quantizes
        parts = [
            jax.device_put(_quantize(img, c, k), devices[c]) for c in range(N_CORES)
        ]
        ii_d = jax.make_array_from_single_device_arrays(
            (N_CORES * CPB, H, W), eng["sh"], parts
        )
        # async dispatch: exec awaits its shards terminal-side
        (out_d,) = fn(ii_d, eng["bands_d"], eng["b16_d"], eng["outdummy_d"][k])

        def fetch(shard, k=k):
            c = shard.index[0].start // CPB if shard.index[0].start else 0
            packed = np.asarray(shard.data)  # [CPB, H, WB] u8
            g0 = c * BPC + k * CPB
            # single gather pass: byte -> 8 f32 pixels, written in place
            view = out[g0 : g0 + CPB, 0].reshape(CPB, H, WB, 8)
            np.take(_bit_lut(), packed, axis=0, out=view)

        futs += [fetch_pool.submit(fetch, s) for s in out_d.addressable_shards]
    for f in futs:
        f.result()
    return out


if __name__ == "__main__":
    import reference as ref

    inputs = ref.setup_inputs()
    out = kernel(**{k: np.asarray(v) for k, v in inputs.items() if k == "img"})
    print("out", out.shape, out.dtype, float(out.sum()))


# revision 24
# speedup vs baseline: 1.0998x; 1.0998x over previous
"""Trainium2 Bass kernel for nn_CannyFilter_BPDA (batch-parallel over 8 cores).

Self-contained: builds a Bass/Tile program per core processing 4 images
(B=32 total / 8 cores), runs via a cached jit(shard_map(bass_exec)) on the
8 axon devices, gathers output.

Host<->device wire is the bottleneck (~45 MB/s through the axon relay), so
the IO contract is minimized:
  - input: channels are summed on the host (the kernel only ever uses
    I = sum_c img_c), so 33.5 MB crosses the wire instead of 100 MB.
  - constants + the dummy output-donation buffer live on device, put once.
  - output: {0,1} edge map is bit-packed on device (8 pixels/byte) and
    expanded with np.unpackbits on the host -> 1 MB instead of 33.5 MB.
  - the jit executable is built once per process and cached.

Algorithm (scale-invariant reformulation of the reference; outputs depend
only on comparisons, so a positive global scale lambda on gx/gy is folded
into thresholds):
  I   = sum_c img_c                    (host)
  Gv  = rho*(I_up + I_dn) + I          (vertical gaussian tap, edge pad)
  Bl  = rho*(Gv_l + Gv_r) + Gv         (horizontal gaussian tap)
  vx  = 0.5*Bl_up + Bl + 0.5*Bl_dn     (PE band matmul, exact 0.5/1 weights)
  wy  = Bl_dn - Bl_up                  (PE band matmul)
  gx  = vx_r - vx_l ; gy = 0.5*(wy_l + wy_r) + wy
  m2  = gx^2 + gy^2 ; mag = sqrt(m2)
  orientation class via |gy| vs tan(k*22.5 deg)*|gx| comparisons
  NMS: keep iff mag > max(neighbor pair along class), neighbors via PE
       shift matmuls (exact fp32r permutation rows)
  z   = keep * ((mag > 0.1*lam) + (mag > 0.3*lam))   in {0,1,2} (= 2*thin)
  hv  = 3x3 box sum of z (exact small ints, bf16/PE)
  out = (z == 2) + (hv >= 2) * (z == 1)   in {0,1}, bit-packed 8px/byte
"""

import math
import os
import sys
from concurrent.futures import ThreadPoolExecutor

for _p in ("/opt/trn_rl_repo", "/opt/pypackages"):
    if os.path.isdir(_p) and _p not in sys.path:
        sys.path.append(_p)

import numpy as np

import concourse.bass as bass
import concourse.bacc as bacc
import concourse.tile as tile
from concourse import mybir
from concourse._compat import with_exitstack
from concourse.mybir import AluOpType as alu

F32 = mybir.dt.float32
U8 = mybir.dt.uint8
F32R = mybir.dt.float32r
BF16 = mybir.dt.bfloat16
AF = mybir.ActivationFunctionType

N_CORES = 8
BPC = 4          # images per core (total)
CPB = 4          # images per core per NEFF call; BPC//CPB pipelined calls
                 # (CPB=2 pipelining tested: the relay serializes transfers
                 # and exec dispatches, so chunking only added latency)
NCHUNK = BPC // CPB
C, H, W = 3, 512, 512
P, NT = 128, 4   # partitions, row-subtiles (H = NT*P)
WB = W // 8      # bit-packed output bytes per row

# ---------------------------------------------------------------- constants
RHO = float(np.float32(math.exp(-0.5)))
_B1D = 1.0 / (1.0 + 2.0 * math.exp(-0.5))       # gaussian 1d center weight
LAM = 3.0 / (_B1D * _B1D)                        # my gx = LAM * ref gx
QF = 65535.0 / 3.0   # host ships I as u16 = round(I*QF); pipeline is
                     # scale-invariant so only the thresholds pick up QF
TLV = float(np.float32(0.1 * LAM * QF))
THV = float(np.float32(0.3 * LAM * QF))
T1V = float(np.float32(math.tan(math.radians(11.25))))
T2V = float(np.float32(math.tan(math.radians(33.75))))

# fp32 band-matrix indices
BI_101_TOP, BI_101_MID, BI_101_BOT = 0, 1, 2
BI_VX_TOP, BI_VX_MID, BI_VX_BOT = 3, 4, 5
BI_WY_TOP, BI_WY_MID, BI_WY_BOT = 6, 7, 8
BI_SSU, BI_SSD = 9, 10
# single-entry cross-tile edge matrices (used as K=64 partition slices):
#   TOPFIX_w: M[127, 0] = w  -> out row 0   += w * src[row127 of t-1]
#   BOTFIX_w: M[0, 127] = w  -> out row 127 += w * src[row0   of t+1]
BI_TOPFIX_1, BI_TOPFIX_05, BI_TOPFIX_N1, BI_BOTFIX_1, BI_BOTFIX_05 = 11, 12, 13, 14, 15
N_BANDS = 16
# bf16 band indices
BI16_111, BI16_TOPFIX_1, BI16_BOTFIX_1 = 0, 1, 2
N_BANDS16 = 3


def make_const_arrays():
    """Band matrices M[k, m]: out[m] = sum_k M[k,m] * in[k]."""
    b101 = np.zeros((3, 128, 128), np.float32)
    for v in range(3):
        for m in range(128):
            if m - 1 >= 0:
                b101[v, m - 1, m] += 1.0
            if m + 1 <= 127:
                b101[v, m + 1, m] += 1.0
    b101[0, 0, 0] += 1.0      # top: replicate row -1 -> row 0
    b101[2, 127, 127] += 1.0  # bottom: replicate

    bvx = np.zeros((3, 128, 128), np.float32)
    for v in range(3):
        for m in range(128):
            bvx[v, m, m] += 1.0
            if m - 1 >= 0:
                bvx[v, m - 1, m] += 0.5
            if m + 1 <= 127:
                bvx[v, m + 1, m] += 0.5
    bvx[0, 0, 0] += 0.5
    bvx[2, 127, 127] += 0.5

    bwy = np.zeros((3, 128, 128), np.float32)
    for v in range(3):
        for m in range(128):
            if m - 1 >= 0:
                bwy[v, m - 1, m] += -1.0
            if m + 1 <= 127:
                bwy[v, m + 1, m] += 1.0
    bwy[0, 0, 0] += -1.0      # top: wy[0] = B[1] - B[0]
    bwy[2, 127, 127] += 1.0   # bottom: wy[127] = B[127] - B[126]

    ssu = np.zeros((128, 128), np.float32)  # out[m] = in[m-1], row0 -> 0
    ssd = np.zeros((128, 128), np.float32)  # out[m] = in[m+1], row127 -> 0
    for m in range(1, 128):
        ssu[m - 1, m] = 1.0
    for m in range(0, 127):
        ssd[m + 1, m] = 1.0

    ee = np.zeros((5, 128, 128), np.float32)
    ee[0, 127, 0] = 1.0    # TOPFIX_1
    ee[1, 127, 0] = 0.5    # TOPFIX_05
    ee[2, 127, 0] = -1.0   # TOPFIX_N1
    ee[3, 0, 127] = 1.0    # BOTFIX_1
    ee[4, 0, 127] = 0.5    # BOTFIX_05

    bands = np.concatenate([b101, bvx, bwy, ssu[None], ssd[None], ee], 0)
    assert bands.shape[0] == N_BANDS

    b16 = np.zeros((N_BANDS16, 128, 128), np.float32)
    for m in range(128):
        b16[BI16_111, m, m] = 1.0
        if m - 1 >= 0:
            b16[BI16_111, m - 1, m] = 1.0
        if m + 1 <= 127:
            b16[BI16_111, m + 1, m] = 1.0
    b16[BI16_TOPFIX_1, 127, 0] = 1.0
    b16[BI16_BOTFIX_1, 0, 127] = 1.0
    import ml_dtypes

    return bands, b16.astype(ml_dtypes.bfloat16)


# ------------------------------------------------------- custom DVE ops
_OPS = {}


def register_custom_ops():
    if _OPS:
        return _OPS
    from concourse import dve_ops as dops
    from concourse.dve_spec import (
        C0,
        C1,
        One,
        Spec,
        Src0,
        Src1,
        Zero,
        _has_src1,
        eq,
        lower,
        maxx,
        sq,
    )
    from concourse.dve_uop import DveOpSpec

    def reg(name, body, reference):
        if name in dops._SUB_OPCODE_FOR_NAME:
            return {o.name: o for o in dops.OPS}[name]
        spec = Spec(body=body, reference=reference)
        row = max(dops._SUB_OPCODE_FOR_NAME.values()) + 1
        assert row < 0x20, "out of custom-DVE opcode rows"
        dops._SUB_OPCODE_FOR_NAME[name] = row
        shas = {}
        for ver in ("v3", "v4"):
            try:
                s = DveOpSpec(
                    name=name,
                    opcode=row,
                    uops=lower(spec, ver=ver),
                    rd1_en=_has_src1(spec),
                )
                shas[ver] = s.sha(ver)
            except Exception:
                pass
        op = dops.DveOp(name, spec, subdim=False, uops_sha=shas)
        dops.OPS.append(op)
        dops.CUSTOM_DVE_SPECS[name] = spec
        return op

    f32 = np.float32

    def _f(x):
        return np.asarray(x, np.float32)

    _OPS["MAG2"] = reg(
        "ANT_CANNY_MAG2",
        sq(Src0) + sq(Src1),
        lambda i0, i1, c0, c1, c2: (_f(i0) * _f(i0)) + (_f(i1) * _f(i1)),
    )
    _OPS["U4A"] = reg(
        "ANT_CANNY_U4A",
        ((Src0 * C0) < Src1) + ((Src0 * C1) < Src1),
        lambda i0, i1, c0, c1, c2: (
            (_f(i0) * f32(c0) < _f(i1)).astype(np.float32)
            + (_f(i0) * f32(c1) < _f(i1)).astype(np.float32)
        ),
    )
    _OPS["U4B"] = reg(
        "ANT_CANNY_U4B",
        ((Src0 * C0) > Src1) + ((Src0 * C1) > Src1),
        lambda i0, i1, c0, c1, c2: (
            (_f(i0) * f32(c0) > _f(i1)).astype(np.float32)
            + (_f(i0) * f32(c1) > _f(i1)).astype(np.float32)
        ),
    )
    _OPS["ZC"] = reg(
        "ANT_CANNY_ZC",
        (Src0 < Src1) * ((Src1 > C0) + (Src1 > C1)),
        lambda i0, i1, c0, c1, c2: (_f(i0) < _f(i1)).astype(np.float32)
        * (
            (_f(i1) > f32(c0)).astype(np.float32)
            + (_f(i1) > f32(c1)).astype(np.float32)
        ),
    )
    _OPS["OUT"] = reg(
        "ANT_CANNY_OUT",
        (Src0 > C0) + ((Src1 > C1) * eq(Src0, One)),
        lambda i0, i1, c0, c1, c2: (_f(i0) > f32(c0)).astype(np.float32)
        + (_f(i1) > f32(c1)).astype(np.float32)
        * (_f(i0) == f32(1.0)).astype(np.float32),
    )
    return _OPS


# ------------------------------------------------------------ kernel body
@with_exitstack
def canny_kernel(ctx, tc, img, bands, b16, outp):
    nc = tc.nc
    V, A, G, T = nc.vector, nc.scalar, nc.gpsimd, nc.tensor
    ops = register_custom_ops()

    sp = ctx.enter_context(tc.tile_pool(name="planes", bufs=1))
    cpool = ctx.enter_context(tc.tile_pool(name="consts", bufs=1))
    pp = ctx.enter_context(tc.tile_pool(name="psum", bufs=1, space="PSUM"))

    # ---- load constants into SBUF
    bt = cpool.tile([128, N_BANDS, 128], F32, tag="bands")
    nc.sync.dma_start(bt[:], bands.rearrange("b k m -> k b m"))
    bt16 = cpool.tile([128, N_BANDS16, 128], BF16, tag="bands16")
    nc.sync.dma_start(bt16[:], b16.rearrange("b k m -> k b m"))

    use_f32r = os.environ.get("CANNY_F32R", "0") == "1"

    def band(i):
        a = bt[:, i, :]
        return a.bitcast(F32R) if use_f32r else a

    def r(ap):
        return ap.bitcast(F32R) if use_f32r else ap

    TRI_SEL = {
        "101": (BI_101_TOP, BI_101_MID, BI_101_MID, BI_101_BOT),
        "vx": (BI_VX_TOP, BI_VX_MID, BI_VX_MID, BI_VX_BOT),
        "wy": (BI_WY_TOP, BI_WY_MID, BI_WY_MID, BI_WY_BOT),
    }
    EDGE_W = {  # (topfix matrix, botfix matrix)
        "101": (BI_TOPFIX_1, BI_BOTFIX_1),
        "vx": (BI_TOPFIX_05, BI_BOTFIX_05),
        "wy": (BI_TOPFIX_N1, BI_BOTFIX_1),
    }

    def run_group(psum_tile, mms):
        for i, (lhsT, rhs) in enumerate(mms):
            T.matmul(
                psum_tile[:], lhsT, rhs, start=(i == 0), stop=(i == len(mms) - 1)
            )

    def tri_matmul(psum_tile, kind, src_plane, t):
        """psum_tile[m,:] = band conv of src_plane subtile t incl cross-tile."""
        etop, ebot = EDGE_W[kind]
        mms = [(band(TRI_SEL[kind][t]), r(src_plane[:, t, :]))]
        if t > 0:
            # out row 0 += w * src[127, t-1] (full-K single-entry matrix)
            mms.append((band(etop), r(src_plane[:, t - 1, :])))
        if t < NT - 1:
            # out row 127 += w * src[0, t+1]
            mms.append((band(ebot), r(src_plane[:, t + 1, :])))
        run_group(psum_tile, mms)

    PARITY_TAGS = {"s1", "s2", "s4", "s5", "s8"}
    par = [0]

    def plane(tag, dtype=F32, shape=None):
        if tag in PARITY_TAGS:
            tag = f"{tag}_{par[0]}"
        return sp.tile([P, NT, W] if shape is None else shape, dtype, tag=tag, name=tag)

    for b in range(CPB):
        par[0] = b % 2
        # ---- load pre-summed u16-quantized intensity, convert to f32
        raw = sp.tile([P, NT, W], mybir.dt.uint16, tag="raw", name="raw")
        nc.sync.dma_start(raw[:], img[b].rearrange("(t p) w -> p t w", p=P))
        ii = plane("s5")
        V.tensor_copy(ii[:], raw[:])

        # ---- vertical gaussian tap: tvI = I_up + I_dn (PE), Gv = rho*tvI + I
        gv = plane("s1")
        for t in range(NT):
            tv = pp.tile([P, W], F32, tag="tv", name="tv")
            tri_matmul(tv, "101", ii, t)
            V.scalar_tensor_tensor(
                gv[:, t, :], tv[:], RHO, ii[:, t, :], alu.mult, alu.add
            )

        # ---- horizontal gaussian tap
        th = plane("s2")
        V.tensor_add(th[:, :, 1:511], gv[:, :, 0:510], gv[:, :, 2:512])
        V.tensor_add(th[:, :, 0:1], gv[:, :, 0:1], gv[:, :, 1:2])
        V.tensor_add(th[:, :, 511:512], gv[:, :, 510:511], gv[:, :, 511:512])
        bl = plane("s3")
        V.scalar_tensor_tensor(bl[:], th[:], RHO, gv[:], alu.mult, alu.add)

        # ---- sobel: vx/wy bands on PE, then horizontal parts
        gx = plane("s1")
        gy = plane("s4")
        th2 = plane("s2")
        vxs = plane("s13")
        wys = plane("s14")
        for t in range(NT):
            vx = pp.tile([P, W], F32, tag="vx", name="vx")
            tri_matmul(vx, "vx", bl, t)
            A.copy(vxs[:, t, :], vx[:])
            V.tensor_sub(gx[:, t, 1:511], vxs[:, t, 2:512], vxs[:, t, 0:510])
            V.tensor_sub(gx[:, t, 0:1], vxs[:, t, 1:2], vxs[:, t, 0:1])
            V.tensor_sub(gx[:, t, 511:512], vxs[:, t, 511:512], vxs[:, t, 510:511])
            wy = pp.tile([P, W], F32, tag="wy", name="wy")
            tri_matmul(wy, "wy", bl, t)
            A.copy(wys[:, t, :], wy[:])
            V.tensor_add(th2[:, t, 1:511], wys[:, t, 0:510], wys[:, t, 2:512])
            V.tensor_add(th2[:, t, 0:1], wys[:, t, 0:1], wys[:, t, 1:2])
            V.tensor_add(th2[:, t, 511:512], wys[:, t, 510:511], wys[:, t, 511:512])
            V.scalar_tensor_tensor(
                gy[:, t, :], th2[:, t, :], 0.5, wys[:, t, :], alu.mult, alu.add
            )

        # ---- magnitude
        m2 = plane("s2")
        V._custom_dve(ops["MAG2"], out=m2[:], in0=gx[:], in1=gy[:])
        mag = plane("s5")
        A.activation(mag[:], m2[:], AF.Sqrt)

        # ---- orientation class count u4 in {0..4}
        ax = plane("s8")
        A.activation(ax[:], gx[:], AF.Abs)
        ay = plane("s9")
        A.activation(ay[:], gy[:], AF.Abs)
        u4a = plane("s2")
        V._custom_dve(ops["U4A"], out=u4a[:], in0=ax[:], in1=ay[:], s0=T1V, s1=T2V)
        u4b = plane("s6")
        V._custom_dve(ops["U4B"], out=u4b[:], in0=ay[:], in1=ax[:], s0=T2V, s1=T1V)
        u4 = plane("s7")
        V.tensor_add(u4[:], u4a[:], u4b[:])
        gp = plane("s2")
        V.tensor_mul(gp[:], gx[:], gy[:])
        spm = plane("s6", U8)
        V.tensor_single_scalar(spm[:], gp[:], 0.0, alu.is_gt)
        m1m = plane("s8", U8)
        V.tensor_single_scalar(m1m[:], u4[:], 1.0, alu.is_equal)
        m2m = plane("s9", U8)
        V.tensor_single_scalar(m2m[:], u4[:], 2.0, alu.is_equal)
        m3m = plane("s10", U8)
        V.tensor_single_scalar(m3m[:], u4[:], 3.0, alu.is_equal)

        # ---- NMS neighbor maxes (P0 doubles as NB selection buffer)
        p0 = plane("s2")
        V.tensor_max(p0[:, :, 1:511], mag[:, :, 0:510], mag[:, :, 2:512])
        A.copy(p0[:, :, 0:1], mag[:, :, 1:2])
        A.copy(p0[:, :, 511:512], mag[:, :, 510:511])
        p1 = plane("s11")
        p2t = plane("s7")
        p3 = plane("s12")
        mus = plane("s15")
        for t in range(NT):
            mu = pp.tile([P, W], F32, tag="shU", name="mu", bufs=2)
            mmu = [(band(BI_SSU), r(mag[:, t, :]))]
            if t > 0:  # row 0 of subtile t is mag row 127 of subtile t-1
                mmu.append((band(BI_TOPFIX_1), r(mag[:, t - 1, :])))
            run_group(mu, mmu)
            A.copy(mus[:, t, :], mu[:])
            md = pp.tile([P, W], F32, tag="shD", name="md", bufs=2)
            mmd = [(band(BI_SSD), r(mag[:, t, :]))]
            if t < NT - 1:
                mmd.append((band(BI_BOTFIX_1), r(mag[:, t + 1, :])))
            run_group(md, mmd)
            V.tensor_max(p2t[:, t, :], mus[:, t, :], md[:])
            V.tensor_max(p1[:, t, 1:511], mus[:, t, 2:512], md[:, 0:510])
            A.copy(p1[:, t, 0:1], mus[:, t, 1:2])
            V.tensor_copy(p1[:, t, 511:512], md[:, 510:511])
            V.tensor_max(p3[:, t, 1:511], mus[:, t, 0:510], md[:, 2:512])
            V.tensor_copy(p3[:, t, 0:1], md[:, 1:2])
            A.copy(p3[:, t, 511:512], mus[:, t, 510:511])

        # ---- diagonal pair selection by gradient sign, then NB by class
        pd1 = plane("s13")
        A.copy(pd1[:], p3[:])
        V.copy_predicated(pd1[:], spm[:], p1[:])
        pd3 = plane("s14")
        A.copy(pd3[:], p1[:])
        V.copy_predicated(pd3[:], spm[:], p3[:])
        V.copy_predicated(p0[:], m1m[:], pd1[:])
        V.copy_predicated(p0[:], m2m[:], p2t[:])
        V.copy_predicated(p0[:], m3m[:], pd3[:])

        # ---- NMS keep + double threshold -> z in {0,1,2} (bf16)
        z = plane("z", BF16)
        V._custom_dve(ops["ZC"], out=z[:], in0=p0[:], in1=mag[:], s0=TLV, s1=THV)

        # ---- hysteresis: 3x3 box sum of z
        hr1 = plane("h1", BF16)
        V.tensor_add(hr1[:, :, 1:511], z[:, :, 0:510], z[:, :, 2:512])
        A.copy(hr1[:, :, 0:1], z[:, :, 1:2])
        A.copy(hr1[:, :, 511:512], z[:, :, 510:511])
        hrow = plane("h2", BF16)
        V.tensor_add(hrow[:], hr1[:], z[:])

        outpl = plane("s1")
        for t in range(NT):
            hv = pp.tile([P, W], F32, tag="hv", name="hv")
            mms = [(bt16[:, BI16_111, :], hrow[:, t, :])]
            if t > 0:
                mms.append((bt16[:, BI16_TOPFIX_1, :], hrow[:, t - 1, :]))
            if t < NT - 1:
                mms.append((bt16[:, BI16_BOTFIX_1, :], hrow[:, t + 1, :]))
            run_group(hv, mms)
            V._custom_dve(
                ops["OUT"], out=outpl[:, t, :], in0=z[:, t, :], in1=hv[:], s0=1.5, s1=1.6
            )

        # ---- bit-pack 8 px/byte (little-endian) and store
        v2 = outpl[:].rearrange("p t (a b) -> p t a b", b=2)
        pk1 = plane("pk1", F32, shape=[P, NT, 256])
        V.scalar_tensor_tensor(
            pk1[:], v2[:, :, :, 1], 2.0, v2[:, :, :, 0], alu.mult, alu.add
        )
        v4 = pk1[:].rearrange("p t (a b) -> p t a b", b=2)
        pk2 = plane("pk2", F32, shape=[P, NT, 128])
        V.scalar_tensor_tensor(
            pk2[:], v4[:, :, :, 1], 4.0, v4[:, :, :, 0], alu.mult, alu.add
        )
        v8 = pk2[:].rearrange("p t (a b) -> p t a b", b=2)
        pk3 = plane("pk3", U8, shape=[P, NT, WB])
        V.scalar_tensor_tensor(
            pk3[:], v8[:, :, :, 1], 16.0, v8[:, :, :, 0], alu.mult, alu.add
        )
        nc.sync.dma_start(outp[b].rearrange("(t p) w -> p t w", p=P), pk3[:])


# --------------------------------------------------------------- build + run
_ENGINE = {}


def build_nc():
    if "nc" in _ENGINE:
        return _ENGINE["nc"]
    register_custom_ops()
    nc = bacc.Bacc("TRN2", target_bir_lowering=False, debug=False)
    img = nc.dram_tensor("img", [CPB, H, W], mybir.dt.uint16, kind="ExternalInput").ap()
    bands = nc.dram_tensor("bands", [N_BANDS, 128, 128], F32, kind="ExternalInput").ap()
    b16 = nc.dram_tensor("b16", [N_BANDS16, 128, 128], BF16, kind="ExternalInput").ap()
    outp = nc.dram_tensor("out", [CPB, H, WB], U8, kind="ExternalOutput").ap()
    with tile.TileContext(nc) as tc:
        canny_kernel(tc, img, bands, b16, outp)
    nc.compile()
    _ENGINE["nc"] = nc
    return nc


def _build_engine():
    """Build (once) the jitted 8-core executable + device-resident operands."""
    if "fn" in _ENGINE:
        return _ENGINE

    import jax
    from jax.sharding import Mesh, NamedSharding, PartitionSpec

    try:
        from jax import shard_map as _shard_map

        def shard_map(f, mesh, in_specs, out_specs):
            return _shard_map(
                f, mesh=mesh, in_specs=in_specs, out_specs=out_specs, check_vma=False
            )
    except ImportError:
        from jax.experimental.shard_map import shard_map as _shard_map_exp

        def shard_map(f, mesh, in_specs, out_specs):
            return _shard_map_exp(
                f, mesh=mesh, in_specs=in_specs, out_specs=out_specs, check_rep=False
            )

    from concourse.bass2jax import (
        _bass_exec_p,
        install_neuronx_cc_hook,
        partition_id_tensor,
    )

    install_neuronx_cc_hook()
    nc = build_nc()

    partition_name = nc.partition_id_tensor.name if nc.partition_id_tensor else None
    in_names, out_names, out_avals = [], [], []
    for alloc in nc.m.functions[0].allocations:
        if not isinstance(alloc, mybir.MemoryLocationSet):
            continue
        name = alloc.memorylocations[0].name
        if alloc.kind == "ExternalInput":
            if name != partition_name:
                in_names.append(name)
        elif alloc.kind == "ExternalOutput":
            out_names.append(name)
            out_avals.append(
                jax.core.ShapedArray(
                    tuple(alloc.tensor_shape), mybir.dt.np(alloc.dtype)
                )
            )
    assert in_names == ["img", "bands", "b16"] and out_names == ["out"], (
        in_names,
        out_names,
    )
    all_in_names = in_names + out_names
    if partition_name is not None:
        all_in_names.append(partition_name)

    def _body(*args):
        operands = list(args)
        if partition_name is not None:
            operands.append(partition_id_tensor())
        outs = _bass_exec_p.bind(
            *operands,
            out_avals=tuple(out_avals),
            in_names=tuple(all_in_names),
            out_names=tuple(out_names),
            lowering_input_output_aliases=(),
            sim_require_finite=True,
            sim_require_nnan=True,
            nc=nc,
        )
        return tuple(outs)

    devices = jax.devices()[:N_CORES]
    assert len(devices) >= N_CORES
    mesh = Mesh(np.asarray(devices), ("core",))
    sh = NamedSharding(mesh, PartitionSpec("core"))
    n_args = len(in_names) + len(out_names)

    def _make_jit():
        return jax.jit(
            shard_map(
                _body,
                mesh=mesh,
                in_specs=(PartitionSpec("core"),) * n_args,
                out_specs=(PartitionSpec("core"),) * len(out_names),
            ),
            keep_unused=True,
        )

    # NOTE: fast_dispatch_compile AOT + LUT-gather unpack were both tried
    # and measured SLOWER here (0.47s vs 0.43s): the fast-dispatch wrapper's
    # per-call python safety-net registration outweighs the saved dispatch
    # on this 1-vCPU host, and np.take gather is 3x slower than
    # unpackbits' sequential C loop.
    fn = _make_jit()

    bands, b16 = make_const_arrays()
    bands_d = jax.device_put(np.concatenate([bands] * N_CORES, 0), sh)
    b16_d = jax.device_put(np.concatenate([b16] * N_CORES, 0), sh)
    # dummy operands for the "out" ExternalOutput slot (kernel writes every
    # byte, so contents never matter); one per in-flight chunk, put once
    outdummy_d = [
        jax.device_put(np.zeros((N_CORES * CPB, H, WB), np.uint8), sh)
        for _ in range(NCHUNK)
    ]
    jax.block_until_ready((bands_d, b16_d, outdummy_d))

    _ENGINE.update(
        fn=fn,
        sh=sh,
        devices=devices,
        bands_d=bands_d,
        b16_d=b16_d,
        outdummy_d=outdummy_d,
        jax=jax,
        put_pool=ThreadPoolExecutor(N_CORES),
        fetch_pool=ThreadPoolExecutor(N_CORES * NCHUNK),
    )
    return _ENGINE


def _quantize(img, c, k):
    """u16 quantized I = floor((sum_c img)*QF) for core c, chunk k.

    3 memory passes (add, add, fused mul+unsafe-cast). Floor instead of
    round shifts the quantization noise mean by step/2 — a constant offset
    on I that the gradient operators cancel, so output flips are unchanged
    in magnitude. Minimizing host passes matters: this box has 1 vCPU
    shared with the websocket proxy that carries the device transfers.
    """
    g0 = c * BPC + k * CPB
    sl = img[g0 : g0 + CPB]
    s = sl[:, 0] + sl[:, 1]
    s += sl[:, 2]
    q = np.empty(s.shape, np.uint16)
    np.multiply(s, np.float32(QF), out=q, casting="unsafe")
    return q


def kernel(**inputs):
    img = np.asarray(inputs["img"])
    assert img.shape == (N_CORES * BPC, C, H, W), img.shape
    if img.dtype != np.float32:
        img = img.astype(np.float32)
    eng = _build_engine()
    jax = eng["jax"]
    devices = eng["devices"]
    fn = eng["fn"]

    out = np.empty((N_CORES * BPC, 1, H, W), np.float32)
    fetch_pool = eng["fetch_pool"]
    futs = []
    for k in range(NCHUNK):
        # sequential per-core quantize+put: on this 1-vCPU host, threads
        # parallelize nothing — front-load the numpy work core by core so
        # each put's async streaming proceeds while the next core quantizes
        parts = [
            jax.device_put(_quantize(img, c, k), devices[c]) for c in range(N_CORES)
        ]
        ii_d = jax.make_array_from_single_device_arrays(
            (N_CORES * CPB, H, W), eng["sh"], parts
        )
        # async dispatch: exec awaits its shards terminal-side
        (out_d,) = fn(ii_d, eng["bands_d"], eng["b16_d"], eng["outdummy_d"][k])

        def fetch(shard, k=k):
            c = shard.index[0].start // CPB if shard.index[0].start else 0
            packed = np.asarray(shard.data)  # [CPB, H, WB] u8
            bits = np.unpackbits(packed, axis=-1, bitorder="little")
            g0 = c * BPC + k * CPB
            out[g0 : g0 + CPB, 0] = bits

        futs += [fetch_pool.submit(fetch, s) for s in out_d.addressable_shards]
    for f in futs:
        f.result()
    return out


if __name__ == "__main__":
    import reference as ref

    inputs = ref.setup_inputs()
    out = kernel(**{k: np.asarray(v) for k, v in inputs.items() if k == "img"})
    print("out", out.shape, out.dtype, float(out.sum()))
